# revision 31
# baseline (speedup 1.0000x reference)
"""Trainium2 Bass kernel for nn_ClassificationAverageModel.

reference:
    pooled = mean(embeddings[x], axis=1)        # (B, D)
    logits = pooled @ W.T + b                   # (B, C)
    out    = softmax(logits, axis=1)

Strategy (memory-regime), v2:
  softmax(mean_w(E[x]) @ W.T + b) == softmax(sum_w((E @ (W.T/L))[x]) + b)
so each core projects its vocab shard down to class space
(P = E_shard @ W.T / L, bf16) and keeps it in SBUF in the dma_gather
"rank-stripe" layout (row r -> partition r%128, 256B segment r//128).

Phase 2 gathers tokens with SBUF-source *transposed* dma_gather
(classes land on partitions, tokens along the free axis), in doc-major
order with a fixed per-doc slot budget, so pooling is a single strided
vector reduce per 512-doc window -- no selection matmuls at all.
SBUF-source avoids the HBM random-read wall that limited the v1 DRAM
gather (~4.8 GB/s/engine under 8-core contention vs ~11 GB/s here).

A ReduceScatter(add) over the [8*20, 512] partial-logit planes gives
each core its 512 docs; 4 PE transposes + bias + softmax finish.

Host-side prep is index bookkeeping only: per (core, doc) local row ids
padded to the budget (pads point at an always-zero table row), laid out
in dma_gather's 16-wrap order, chunked into <=896-index calls (the
single-packet descriptor-ring cap for transpose mode).
"""

import numpy as np

import concourse.bass as bass
import concourse.mybir as mybir
import concourse.tile as tile
from concourse import bacc, library_config
from concourse.bass_utils import run_bass_kernel_spmd
from concourse.masks import make_identity
from concourse.vector_clock import ScopedClock

F32 = mybir.dt.float32
BF16 = mybir.dt.bfloat16
I16 = mybir.dt.int16

NCORES = 8
# max idxs per single-packet gather call: transpose mode needs
# num_idxs/16 + 2 descriptors per engine ring, capped at 896.
# (single_packet=False fires the completion sem before the rx transpose
# sprays land -- measured corruption -- so stay in single-packet mode.)
GSUB = 896


class PatchedTileContext(tile.TileContext):
    """Split the kernel-tail drain's sem waits: walrus TRN2 CTRL codegen
    rejects drain instructions carrying more than ~2 sync waits."""

    def _drain_and_barrier(self, tick_clock, wait_clock):
        drain_inst = self.nc.sync.drain()
        wait_clock.add_sem_waits(
            drain_inst.ins, ScopedClock({None: tick_clock.global_clock})
        )
        si = drain_inst.ins.sync_info
        waits = list(si.on_wait) if si is not None else []
        if len(waits) > 1:
            si.on_wait = waits[:1]
            for w in waits[1:]:
                d2 = self.nc.sync.drain()
                si2 = d2.ins.sync_info
                if si2 is None:
                    d2.ins.sync_info = mybir.SyncInfo(on_wait=[w], on_update=[])
                else:
                    si2.on_wait = [w]
        self.nc.all_engine_barrier()
        popped = self.nc._tile_sem_poison_stack.pop()
        assert popped is self._sem_poison
        self.nc.clear_and_free_semaphores(list(self.sems.allocated().values()))
        self.nc.all_engine_barrier()


class Cfg:
    def __init__(self, vocab=100000, embed=300, ncls=20, batch=4096, doclen=200,
                 budget=44, wdocs=512):
        assert vocab % NCORES == 0 and batch % (128 * NCORES) == 0
        self.vocab, self.embed, self.ncls = vocab, embed, ncls
        self.batch, self.doclen = batch, doclen
        self.vsh = vocab // NCORES                  # shard rows per core
        self.nch = -(-self.vsh // 128)              # 128-row chunks
        self.tranks = -(-(self.nch * 128) // 128)   # segments used by P
        # table segments: P segments + 1 spare holding the zero pad row
        self.tsegs = self.nch + 1
        self.pad_idx = self.nch * 128               # row in the spare segment
        self.trows = self.tsegs * 128
        self.budget = budget                        # token slots per doc
        # gather/reduce group: gdocs docs in <=2 calls so the per-group
        # reduce carries at most 2 producer sem-waits (walrus codegen limit)
        self.gdocs = None
        for gd in (32, 16, 8, 4):
            if gd * budget <= 2 * GSUB and (gd * budget) % 128 == 0:
                self.gdocs = gd
                break
        assert self.gdocs, f"no group size for budget {budget}"
        gtok = self.gdocs * budget
        if gtok <= GSUB:
            self.gcalls = [gtok]
        else:
            self.gcalls = [GSUB, gtok - GSUB]
        assert all(n % 128 == 0 and 0 < n <= GSUB for n in self.gcalls)
        self.gtok = gtok
        assert batch % self.gdocs == 0
        self.ngrp = batch // self.gdocs
        self.docs_out = batch // NCORES
        self.kchunks = [(0, 128), (128, 128), (256, 44)]

    def key(self):
        return (self.vocab, self.embed, self.ncls, self.batch, self.doclen,
                self.budget, self.gdocs)


def _build_program(cfg: Cfg):
    c = cfg
    nc = bacc.Bacc("TRN2", target_bir_lowering=False, debug=False,
                   num_devices=NCORES, num_swdge_queues=4)
    e_sh = nc.dram_tensor("e_sh", [c.vsh, c.embed], F32, kind="ExternalInput")
    w_in = nc.dram_tensor("w_in", [c.ncls, c.embed], F32, kind="ExternalInput")
    b_in = nc.dram_tensor("b_in", [128, c.ncls], F32, kind="ExternalInput")
    gidx = nc.dram_tensor("gidx", [128, c.batch * c.budget // 16], I16,
                          kind="ExternalInput")
    out = nc.dram_tensor("out", [c.docs_out, c.ncls], F32,
                         kind="ExternalOutput")
    partials_d = nc.dram_tensor("partials_d", [NCORES * c.ncls, c.docs_out], F32)
    rs_d = nc.dram_tensor("rs_d", [c.ncls, c.docs_out], F32)

    nk = len(c.kchunks)
    with PatchedTileContext(nc) as tc:
        with tc.tile_pool(name="const", bufs=1) as cpool:
            nc.gpsimd.load_library(library_config.mlp)

            ident = cpool.tile([128, 128], F32)
            make_identity(nc, ident[:])
            ident_b = cpool.tile([128, 128], BF16)
            nc.vector.tensor_copy(out=ident_b[:], in_=ident[:])

            b_t = cpool.tile([128, c.ncls], F32)
            nc.sync.dma_start(out=b_t[:], in_=b_in[:])

            # ---- Wt = W.T / doclen, bf16, one [128, ncls] tile per k-chunk
            w_sb = cpool.tile([128, c.embed], F32)
            nc.sync.dma_start(out=w_sb[:c.ncls, :], in_=w_in[:])
            wt = cpool.tile([128, nk * c.ncls], BF16)
            nc.vector.memset(wt[:], 0.0)
            with tc.tile_pool(name="wps", bufs=nk, space="PSUM") as wps:
                for k, (k0, kw) in enumerate(c.kchunks):
                    kreal = min(kw, c.embed - k0)
                    wt_ps = wps.tile([128, 128], F32)
                    nc.tensor.transpose(
                        out=wt_ps[:kreal, :c.ncls],
                        in_=w_sb[:c.ncls, k0:k0 + kreal],
                        identity=ident[:c.ncls, :c.ncls],
                    )
                    nc.scalar.mul(
                        out=wt[:kreal, k * c.ncls:(k + 1) * c.ncls],
                        in_=wt_ps[:kreal, :c.ncls],
                        mul=1.0 / c.doclen,
                    )

            # ---- the projected table, rank-stripe layout ----
            t_sb = cpool.tile([128, c.trows], BF16)
            # zero the whole table: pads gather from the spare segment, and
            # elems ncls:128 of every segment flow into pooled rows >= ncls
            # (never consumed, but keep them finite / sim-checkable)
            nc.vector.memset(t_sb[:], 0.0)

            # ---- phase 1: P chunks = (E.T chunk).T @ Wt ----
            # E chunk -> bf16 -> PE transpose (bf16, via identity) -> PSUM
            # -> bf16 copy -> lhsT for the projection matmul.
            with (
                tc.tile_pool(name="ep", bufs=3) as epool,
                tc.tile_pool(name="eb", bufs=3) as ebpool,
                tc.tile_pool(name="et", bufs=6) as etpool,
                tc.tile_pool(name="tps", bufs=4, space="PSUM") as tpool,
                tc.tile_pool(name="pps", bufs=4, space="PSUM") as ppool,
            ):
                for ch in range(c.nch):
                    r0 = ch * 128
                    rows = min(128, c.vsh - r0)
                    e_t = epool.tile([128, c.embed], F32)
                    nc.sync.dma_start(out=e_t[:rows, :], in_=e_sh[r0:r0 + rows, :])
                    e_b = ebpool.tile([128, c.embed], BF16)
                    nc.vector.tensor_copy(out=e_b[:rows, :], in_=e_t[:rows, :])
                    pp = ppool.tile([128, c.ncls], F32)
                    ets = []
                    for k, (k0, kw) in enumerate(c.kchunks):
                        tp = tpool.tile([128, 128], BF16)
                        nc.tensor.transpose(
                            out=tp[:kw, :rows],
                            in_=e_b[:rows, k0:k0 + kw],
                            identity=ident_b[:rows, :rows],
                        )
                        et_k = etpool.tile([128, 128], BF16)
                        nc.scalar.copy(out=et_k[:kw, :rows], in_=tp[:kw, :rows])
                        ets.append(et_k)
                    for k, (k0, kw) in enumerate(c.kchunks):
                        nc.tensor.matmul(
                            out=pp[:rows, :],
                            lhsT=ets[k][:kw, :rows],
                            rhs=wt[:kw, k * c.ncls:(k + 1) * c.ncls],
                            start=(k == 0),
                            stop=(k == nk - 1),
                        )
                    nc.vector.tensor_copy(
                        out=t_sb[:rows, ch * 128:ch * 128 + c.ncls],
                        in_=pp[:rows, :])

            # ---- phase 2: transposed SBUF gather + per-group reduce ----
            pooled = cpool.tile([128, c.batch], F32)
            gi_all = cpool.tile([128, c.batch * c.budget // 16], I16)
            nc.sync.dma_start(out=gi_all[:], in_=gidx[:])
            with tc.tile_pool(name="gw", bufs=4) as gwpool:
                qn = 0
                for grp in range(c.ngrp):
                    base = grp * c.gtok
                    g_w = gwpool.tile([128, c.gtok], BF16)
                    g3 = g_w[:].rearrange("p (s n) -> p s n", s=1)
                    # alternate call order so each of the 4 SWDGE queues gets
                    # an equal share of tokens (desc-gen is ~8ns/token on the
                    # queue's Q7 pair; a fixed order starves two queues)
                    calls = list(c.gcalls)
                    if (grp >> 1) & 1:
                        calls.reverse()
                    off = 0
                    for n in calls:
                        nc.gpsimd.dma_gather(
                            out_ap=g3[:, :, off:off + n],
                            in_ap=t_sb[:],
                            idxs_ap=gi_all[:, (base + off) // 16:
                                           (base + off + n) // 16],
                            num_idxs=n,
                            num_idxs_reg=n,
                            elem_size=128,
                            transpose=True,
                            single_packet=True,
                            queue_num=qn % 4,
                            sbuf_tokens_per_rank=128,
                            sbuf_free_dim_per_rank=256,
                            sbuf_free_dim_pad_per_rank=0,
                            sbuf_byte_offset=0,
                        )
                        off += n
                        qn += 1
                    g3d = g_w[:].rearrange("p (d t) -> p d t", t=c.budget)
                    nc.vector.tensor_reduce(
                        out=pooled[:, grp * c.gdocs:(grp + 1) * c.gdocs],
                        in_=g3d,
                        op=mybir.AluOpType.add,
                        axis=mybir.AxisListType.X)

            # ---- phase 3: RS + bias + softmax ----
            for g in range(NCORES):
                nc.sync.dma_start(
                    out=partials_d[g * c.ncls:(g + 1) * c.ncls, :],
                    in_=pooled[:c.ncls, g * c.docs_out:(g + 1) * c.docs_out])
            nc.gpsimd.collective_compute(
                "ReduceScatter",
                mybir.AluOpType.add,
                replica_groups=[list(range(NCORES))],
                ins=[partials_d[:]],
                outs=[rs_d[:]],
            )
            with (
                tc.tile_pool(name="sm", bufs=2) as smpool,
                tc.tile_pool(name="sms", bufs=2) as sspool,
                tc.tile_pool(name="tps", bufs=2, space="PSUM") as tpool,
            ):
                rs_sb = cpool.tile([c.ncls, c.docs_out], F32)
                nc.sync.dma_start(out=rs_sb[:], in_=rs_d[:])
                for t in range(c.docs_out // 128):
                    tp = tpool.tile([128, c.ncls], F32)
                    nc.tensor.transpose(
                        out=tp[:, :c.ncls],
                        in_=rs_sb[:, t * 128:(t + 1) * 128],
                        identity=ident[:c.ncls, :c.ncls],
                    )
                    lt = smpool.tile([128, c.ncls], F32)
                    nc.vector.tensor_tensor(out=lt[:], in0=tp[:], in1=b_t[:],
                                            op=mybir.AluOpType.add)
                    nmx = sspool.tile([128, 1], F32)
                    nc.vector.tensor_reduce(out=nmx[:], in_=lt[:],
                                            op=mybir.AluOpType.max,
                                            axis=mybir.AxisListType.X,
                                            negate=True)
                    ex = smpool.tile([128, c.ncls], F32)
                    nc.scalar.activation(out=ex[:], in_=lt[:],
                                         func=mybir.ActivationFunctionType.Exp,
                                         bias=nmx[:], scale=1.0)
                    sm = sspool.tile([128, 1], F32)
                    nc.vector.reduce_sum(out=sm[:], in_=ex[:],
                                         axis=mybir.AxisListType.X)
                    rc = sspool.tile([128, 1], F32)
                    nc.vector.reciprocal(out=rc[:], in_=sm[:])
                    ot = smpool.tile([128, c.ncls], F32)
                    nc.vector.tensor_scalar_mul(out=ot[:], in0=ex[:],
                                                scalar1=rc[:])
                    nc.sync.dma_start(out=out[t * 128:(t + 1) * 128, :],
                                      in_=ot[:])
    nc.compile()
    return nc


def _prep_index_inputs(cfg: Cfg, x: np.ndarray):
    """Doc-major gather indices (16-wrap int16 per call).
    Returns (gidx[8, 128, B*budget/16], max_count)."""
    c = cfg
    x = x.astype(np.int64)
    flat_v = x.reshape(-1)
    tok_doc = np.repeat(np.arange(c.batch, dtype=np.int64), c.doclen)
    core_of = flat_v // c.vsh
    local = (flat_v - core_of * c.vsh).astype(np.int32)

    key = core_of * c.batch + tok_doc
    counts = np.bincount(key, minlength=NCORES * c.batch)
    max_count = int(counts.max())
    if max_count > c.budget:
        return None, max_count
    order = np.argsort(key, kind="stable")
    key_s = key[order]
    group_start = np.zeros(NCORES * c.batch, np.int64)
    np.cumsum(counts[:-1], out=group_start[1:])
    pos = np.arange(key.size, dtype=np.int64) - group_start[key_s]
    slot = (key_s % c.batch) * c.budget + pos
    core_s = key_s // c.batch

    nslots = c.batch * c.budget
    # pads round-robin over the 128 zero rows of the spare segment, so pad
    # reads spread across all SBUF partitions instead of hammering one port
    gflat = np.broadcast_to(
        c.pad_idx + (np.arange(nslots, dtype=np.int32) % 128),
        (NCORES, nslots)).copy()
    gflat[core_s, slot] = local[order]

    # 16-wrap per call: within each call, token j -> [j%16, j//16]
    # (call order alternates per group to balance SWDGE queues -- must
    # match the build loop exactly)
    call_sizes = []
    for grp in range(c.ngrp):
        calls = list(c.gcalls)
        if (grp >> 1) & 1:
            calls.reverse()
        call_sizes.extend(calls)
    g16 = np.empty((NCORES, 16, nslots // 16), np.int16)
    off = 0
    for n in call_sizes:
        seg = gflat[:, off:off + n].reshape(NCORES, n // 16, 16)
        g16[:, :, off // 16:(off + n) // 16] = seg.transpose(0, 2, 1)
        off += n
    gidx = np.tile(g16, (1, 8, 1)).astype(np.int16)   # (8, 128, cols)
    return gidx, max_count


_PROGRAM_CACHE: dict = {}


def _get_program(cfg: Cfg):
    k = cfg.key()
    if k not in _PROGRAM_CACHE:
        _PROGRAM_CACHE[k] = _build_program(cfg)
    return _PROGRAM_CACHE[k]


def run(embeddings, W, b, x, cfg: Cfg | None = None, trace=False, tmpdir=None):
    if cfg is None:
        cfg = Cfg()
    embeddings = np.ascontiguousarray(np.asarray(embeddings, dtype=np.float32))
    W = np.ascontiguousarray(np.asarray(W, dtype=np.float32))
    b = np.asarray(b, dtype=np.float32).reshape(1, -1)
    x = np.asarray(x)

    gidx, max_count = _prep_index_inputs(cfg, x)
    while gidx is None:  # budget overflow (non-uniform input): grow and retry
        bigger = -(-max_count // 4) * 4
        while True:
            try:
                cfg = Cfg(cfg.vocab, cfg.embed, cfg.ncls, cfg.batch,
                          cfg.doclen, budget=bigger)
                break
            except AssertionError:
                bigger += 4
        gidx, max_count = _prep_index_inputs(cfg, x)

    nc = _get_program(cfg)
    b_tiled = np.tile(b, (128, 1))
    in_maps = []
    for c in range(NCORES):
        in_maps.append({
            "e_sh": embeddings[c * cfg.vsh:(c + 1) * cfg.vsh],
            "w_in": W,
            "b_in": b_tiled,
            "gidx": gidx[c],
        })
    res = run_bass_kernel_spmd(nc, in_maps, list(range(NCORES)),
                               trace=trace, tmpdir=tmpdir)
    out = np.concatenate([res.results[c]["out"] for c in range(NCORES)],
                         axis=0)
    return out, res


def kernel(embeddings, W, b, x):
    out, _ = run(embeddings, W, b, x)
    return out


# revision 32
# speedup vs baseline: 1.0990x; 1.0990x over previous
"""Trainium2 Bass kernel for nn_ClassificationAverageModel.

reference:
    pooled = mean(embeddings[x], axis=1)        # (B, D)
    logits = pooled @ W.T + b                   # (B, C)
    out    = softmax(logits, axis=1)

Strategy (memory-regime), v2:
  softmax(mean_w(E[x]) @ W.T + b) == softmax(sum_w((E @ (W.T/L))[x]) + b)
so each core projects its vocab shard down to class space
(P = E_shard @ W.T / L, bf16) and keeps it in SBUF in the dma_gather
"rank-stripe" layout (row r -> partition r%128, 256B segment r//128).

Phase 2 gathers tokens with SBUF-source *transposed* dma_gather
(classes land on partitions, tokens along the free axis), in doc-major
order with a fixed per-doc slot budget, so pooling is a single strided
vector reduce per 512-doc window -- no selection matmuls at all.
SBUF-source avoids the HBM random-read wall that limited the v1 DRAM
gather (~4.8 GB/s/engine under 8-core contention vs ~11 GB/s here).

A ReduceScatter(add) over the [8*20, 512] partial-logit planes gives
each core its 512 docs; 4 PE transposes + bias + softmax finish.

Host-side prep is index bookkeeping only: per (core, doc) local row ids
padded to the budget (pads point at an always-zero table row), laid out
in dma_gather's 16-wrap order, chunked into <=896-index calls (the
single-packet descriptor-ring cap for transpose mode).
"""

import numpy as np

import concourse.bass as bass
import concourse.mybir as mybir
import concourse.tile as tile
from concourse import bacc, library_config
from concourse.bass_utils import run_bass_kernel_spmd
from concourse.masks import make_identity
from concourse.vector_clock import ScopedClock

F32 = mybir.dt.float32
BF16 = mybir.dt.bfloat16
I16 = mybir.dt.int16

NCORES = 8
# max idxs per single-packet gather call: transpose mode needs
# num_idxs/16 + 2 descriptors per engine ring, capped at 896.
# (single_packet=False fires the completion sem before the rx transpose
# sprays land -- measured corruption -- so stay in single-packet mode.)
GSUB = 896


class PatchedTileContext(tile.TileContext):
    """Split the kernel-tail drain's sem waits: walrus TRN2 CTRL codegen
    rejects drain instructions carrying more than ~2 sync waits."""

    def _drain_and_barrier(self, tick_clock, wait_clock):
        drain_inst = self.nc.sync.drain()
        wait_clock.add_sem_waits(
            drain_inst.ins, ScopedClock({None: tick_clock.global_clock})
        )
        si = drain_inst.ins.sync_info
        waits = list(si.on_wait) if si is not None else []
        if len(waits) > 1:
            si.on_wait = waits[:1]
            for w in waits[1:]:
                d2 = self.nc.sync.drain()
                si2 = d2.ins.sync_info
                if si2 is None:
                    d2.ins.sync_info = mybir.SyncInfo(on_wait=[w], on_update=[])
                else:
                    si2.on_wait = [w]
        self.nc.all_engine_barrier()
        popped = self.nc._tile_sem_poison_stack.pop()
        assert popped is self._sem_poison
        self.nc.clear_and_free_semaphores(list(self.sems.allocated().values()))
        self.nc.all_engine_barrier()


class Cfg:
    def __init__(self, vocab=100000, embed=300, ncls=20, batch=4096, doclen=200,
                 budget=44, wdocs=512):
        assert vocab % NCORES == 0 and batch % (128 * NCORES) == 0
        self.vocab, self.embed, self.ncls = vocab, embed, ncls
        self.batch, self.doclen = batch, doclen
        self.vsh = vocab // NCORES                  # shard rows per core
        self.nch = -(-self.vsh // 128)              # 128-row chunks
        self.tranks = -(-(self.nch * 128) // 128)   # segments used by P
        # table segments: P segments + 1 spare holding the zero pad row
        self.tsegs = self.nch + 1
        self.pad_idx = self.nch * 128               # row in the spare segment
        self.trows = self.tsegs * 128
        self.budget = budget                        # token slots per doc
        # gather/reduce group: gdocs docs in <=2 calls so the per-group
        # reduce carries at most 2 producer sem-waits (walrus codegen limit)
        self.gdocs = None
        for gd in (32, 16, 8, 4):
            if gd * budget <= 2 * GSUB and (gd * budget) % 128 == 0:
                self.gdocs = gd
                break
        assert self.gdocs, f"no group size for budget {budget}"
        gtok = self.gdocs * budget
        if gtok <= GSUB:
            self.gcalls = [gtok]
        else:
            self.gcalls = [GSUB, gtok - GSUB]
        assert all(n % 128 == 0 and 0 < n <= GSUB for n in self.gcalls)
        self.gtok = gtok
        assert batch % self.gdocs == 0
        self.ngrp = batch // self.gdocs
        self.docs_out = batch // NCORES
        self.kchunks = [(0, 128), (128, 128), (256, 44)]

    def key(self):
        return (self.vocab, self.embed, self.ncls, self.batch, self.doclen,
                self.budget, self.gdocs)


def _build_program(cfg: Cfg):
    c = cfg
    nc = bacc.Bacc("TRN2", target_bir_lowering=False, debug=False,
                   num_devices=NCORES, num_swdge_queues=4)
    e_sh = nc.dram_tensor("e_sh", [c.vsh, c.embed], F32, kind="ExternalInput")
    w_in = nc.dram_tensor("w_in", [c.ncls, c.embed], F32, kind="ExternalInput")
    b_in = nc.dram_tensor("b_in", [128, c.ncls], F32, kind="ExternalInput")
    gidx = nc.dram_tensor("gidx", [128, c.batch * c.budget // 16], I16,
                          kind="ExternalInput")
    out = nc.dram_tensor("out", [c.docs_out, c.ncls], F32,
                         kind="ExternalOutput")
    partials_d = nc.dram_tensor("partials_d", [NCORES * c.ncls, c.docs_out], F32)
    rs_d = nc.dram_tensor("rs_d", [c.ncls, c.docs_out], F32)

    nk = len(c.kchunks)
    with PatchedTileContext(nc) as tc:
        with tc.tile_pool(name="const", bufs=1) as cpool:
            nc.gpsimd.load_library(library_config.mlp)

            ident = cpool.tile([128, 128], F32)
            make_identity(nc, ident[:])
            ident_b = cpool.tile([128, 128], BF16)
            nc.vector.tensor_copy(out=ident_b[:], in_=ident[:])

            b_t = cpool.tile([128, c.ncls], F32)
            nc.sync.dma_start(out=b_t[:], in_=b_in[:])

            # ---- Wt = W.T / doclen, bf16, one [128, ncls] tile per k-chunk
            w_sb = cpool.tile([128, c.embed], F32)
            nc.sync.dma_start(out=w_sb[:c.ncls, :], in_=w_in[:])
            wt = cpool.tile([128, nk * c.ncls], BF16)
            nc.vector.memset(wt[:], 0.0)
            with tc.tile_pool(name="wps", bufs=nk, space="PSUM") as wps:
                for k, (k0, kw) in enumerate(c.kchunks):
                    kreal = min(kw, c.embed - k0)
                    wt_ps = wps.tile([128, 128], F32)
                    nc.tensor.transpose(
                        out=wt_ps[:kreal, :c.ncls],
                        in_=w_sb[:c.ncls, k0:k0 + kreal],
                        identity=ident[:c.ncls, :c.ncls],
                    )
                    nc.scalar.mul(
                        out=wt[:kreal, k * c.ncls:(k + 1) * c.ncls],
                        in_=wt_ps[:kreal, :c.ncls],
                        mul=1.0 / c.doclen,
                    )

            # ---- the projected table, rank-stripe layout ----
            t_sb = cpool.tile([128, c.trows], BF16)
            # zero the whole table: pads gather from the spare segment, and
            # elems ncls:128 of every segment flow into pooled rows >= ncls
            # (never consumed, but keep them finite / sim-checkable)
            nc.vector.memset(t_sb[:], 0.0)

            # ---- phase 1: P chunks = (E.T chunk).T @ Wt ----
            # E chunk -> bf16 -> PE transpose (bf16, via identity) -> PSUM
            # -> bf16 copy -> lhsT for the projection matmul.
            with (
                tc.tile_pool(name="ep", bufs=3) as epool,
                tc.tile_pool(name="eb", bufs=3) as ebpool,
                tc.tile_pool(name="et", bufs=6) as etpool,
                tc.tile_pool(name="tps", bufs=4, space="PSUM") as tpool,
                tc.tile_pool(name="pps", bufs=4, space="PSUM") as ppool,
            ):
                for ch in range(c.nch):
                    r0 = ch * 128
                    rows = min(128, c.vsh - r0)
                    e_t = epool.tile([128, c.embed], F32)
                    nc.sync.dma_start(out=e_t[:rows, :], in_=e_sh[r0:r0 + rows, :])
                    pp = ppool.tile([128, c.ncls], F32)
                    ets = []
                    for k, (k0, kw) in enumerate(c.kchunks):
                        tp = tpool.tile([128, 128], F32)
                        nc.tensor.transpose(
                            out=tp[:kw, :rows],
                            in_=e_t[:rows, k0:k0 + kw],
                            identity=ident[:rows, :rows],
                        )
                        et_k = etpool.tile([128, 128], BF16)
                        nc.scalar.copy(out=et_k[:kw, :rows], in_=tp[:kw, :rows])
                        ets.append(et_k)
                    for k, (k0, kw) in enumerate(c.kchunks):
                        nc.tensor.matmul(
                            out=pp[:rows, :],
                            lhsT=ets[k][:kw, :rows],
                            rhs=wt[:kw, k * c.ncls:(k + 1) * c.ncls],
                            start=(k == 0),
                            stop=(k == nk - 1),
                        )
                    nc.vector.tensor_copy(
                        out=t_sb[:rows, ch * 128:ch * 128 + c.ncls],
                        in_=pp[:rows, :])

            # ---- phase 2: transposed SBUF gather + per-group reduce ----
            pooled = cpool.tile([128, c.batch], F32)
            gi_all = cpool.tile([128, c.batch * c.budget // 16], I16)
            nc.sync.dma_start(out=gi_all[:], in_=gidx[:])
            with tc.tile_pool(name="gw", bufs=4) as gwpool:
                qn = 0
                for grp in range(c.ngrp):
                    base = grp * c.gtok
                    g_w = gwpool.tile([128, c.gtok], BF16)
                    g3 = g_w[:].rearrange("p (s n) -> p s n", s=1)
                    # alternate call order so each of the 4 SWDGE queues gets
                    # an equal share of tokens (desc-gen is ~8ns/token on the
                    # queue's Q7 pair; a fixed order starves two queues)
                    calls = list(c.gcalls)
                    if (grp >> 1) & 1:
                        calls.reverse()
                    off = 0
                    for n in calls:
                        nc.gpsimd.dma_gather(
                            out_ap=g3[:, :, off:off + n],
                            in_ap=t_sb[:],
                            idxs_ap=gi_all[:, (base + off) // 16:
                                           (base + off + n) // 16],
                            num_idxs=n,
                            num_idxs_reg=n,
                            elem_size=128,
                            transpose=True,
                            single_packet=True,
                            queue_num=qn % 4,
                            sbuf_tokens_per_rank=128,
                            sbuf_free_dim_per_rank=256,
                            sbuf_free_dim_pad_per_rank=0,
                            sbuf_byte_offset=0,
                        )
                        off += n
                        qn += 1
                    g3d = g_w[:].rearrange("p (d t) -> p d t", t=c.budget)
                    nc.vector.tensor_reduce(
                        out=pooled[:, grp * c.gdocs:(grp + 1) * c.gdocs],
                        in_=g3d,
                        op=mybir.AluOpType.add,
                        axis=mybir.AxisListType.X)

            # ---- phase 3: RS + bias + softmax ----
            for g in range(NCORES):
                nc.sync.dma_start(
                    out=partials_d[g * c.ncls:(g + 1) * c.ncls, :],
                    in_=pooled[:c.ncls, g * c.docs_out:(g + 1) * c.docs_out])
            nc.gpsimd.collective_compute(
                "ReduceScatter",
                mybir.AluOpType.add,
                replica_groups=[list(range(NCORES))],
                ins=[partials_d[:]],
                outs=[rs_d[:]],
            )
            with (
                tc.tile_pool(name="sm", bufs=2) as smpool,
                tc.tile_pool(name="sms", bufs=2) as sspool,
                tc.tile_pool(name="tps", bufs=2, space="PSUM") as tpool,
            ):
                rs_sb = cpool.tile([c.ncls, c.docs_out], F32)
                nc.sync.dma_start(out=rs_sb[:], in_=rs_d[:])
                for t in range(c.docs_out // 128):
                    tp = tpool.tile([128, c.ncls], F32)
                    nc.tensor.transpose(
                        out=tp[:, :c.ncls],
                        in_=rs_sb[:, t * 128:(t + 1) * 128],
                        identity=ident[:c.ncls, :c.ncls],
                    )
                    lt = smpool.tile([128, c.ncls], F32)
                    nc.vector.tensor_tensor(out=lt[:], in0=tp[:], in1=b_t[:],
                                            op=mybir.AluOpType.add)
                    nmx = sspool.tile([128, 1], F32)
                    nc.vector.tensor_reduce(out=nmx[:], in_=lt[:],
                                            op=mybir.AluOpType.max,
                                            axis=mybir.AxisListType.X,
                                            negate=True)
                    ex = smpool.tile([128, c.ncls], F32)
                    nc.scalar.activation(out=ex[:], in_=lt[:],
                                         func=mybir.ActivationFunctionType.Exp,
                                         bias=nmx[:], scale=1.0)
                    sm = sspool.tile([128, 1], F32)
                    nc.vector.reduce_sum(out=sm[:], in_=ex[:],
                                         axis=mybir.AxisListType.X)
                    rc = sspool.tile([128, 1], F32)
                    nc.vector.reciprocal(out=rc[:], in_=sm[:])
                    ot = smpool.tile([128, c.ncls], F32)
                    nc.vector.tensor_scalar_mul(out=ot[:], in0=ex[:],
                                                scalar1=rc[:])
                    nc.sync.dma_start(out=out[t * 128:(t + 1) * 128, :],
                                      in_=ot[:])
    nc.compile()
    return nc


def _prep_index_inputs(cfg: Cfg, x: np.ndarray):
    """Doc-major gather indices (16-wrap int16 per call).
    Returns (gidx[8, 128, B*budget/16], max_count)."""
    c = cfg
    x = x.astype(np.int64)
    flat_v = x.reshape(-1)
    tok_doc = np.repeat(np.arange(c.batch, dtype=np.int64), c.doclen)
    core_of = flat_v // c.vsh
    local = (flat_v - core_of * c.vsh).astype(np.int32)

    key = core_of * c.batch + tok_doc
    counts = np.bincount(key, minlength=NCORES * c.batch)
    max_count = int(counts.max())
    if max_count > c.budget:
        return None, max_count
    order = np.argsort(key, kind="stable")
    key_s = key[order]
    group_start = np.zeros(NCORES * c.batch, np.int64)
    np.cumsum(counts[:-1], out=group_start[1:])
    pos = np.arange(key.size, dtype=np.int64) - group_start[key_s]
    slot = (key_s % c.batch) * c.budget + pos
    core_s = key_s // c.batch

    nslots = c.batch * c.budget
    # pads round-robin over the 128 zero rows of the spare segment, so pad
    # reads spread across all SBUF partitions instead of hammering one port
    gflat = np.broadcast_to(
        c.pad_idx + (np.arange(nslots, dtype=np.int32) % 128),
        (NCORES, nslots)).copy()
    gflat[core_s, slot] = local[order]

    # 16-wrap per call: within each call, token j -> [j%16, j//16]
    # (call order alternates per group to balance SWDGE queues -- must
    # match the build loop exactly)
    call_sizes = []
    for grp in range(c.ngrp):
        calls = list(c.gcalls)
        if (grp >> 1) & 1:
            calls.reverse()
        call_sizes.extend(calls)
    g16 = np.empty((NCORES, 16, nslots // 16), np.int16)
    off = 0
    for n in call_sizes:
        seg = gflat[:, off:off + n].reshape(NCORES, n // 16, 16)
        g16[:, :, off // 16:(off + n) // 16] = seg.transpose(0, 2, 1)
        off += n
    gidx = np.tile(g16, (1, 8, 1)).astype(np.int16)   # (8, 128, cols)
    return gidx, max_count


_PROGRAM_CACHE: dict = {}


def _get_program(cfg: Cfg):
    k = cfg.key()
    if k not in _PROGRAM_CACHE:
        _PROGRAM_CACHE[k] = _build_program(cfg)
    return _PROGRAM_CACHE[k]


def run(embeddings, W, b, x, cfg: Cfg | None = None, trace=False, tmpdir=None):
    if cfg is None:
        cfg = Cfg()
    embeddings = np.ascontiguousarray(np.asarray(embeddings, dtype=np.float32))
    W = np.ascontiguousarray(np.asarray(W, dtype=np.float32))
    b = np.asarray(b, dtype=np.float32).reshape(1, -1)
    x = np.asarray(x)

    gidx, max_count = _prep_index_inputs(cfg, x)
    while gidx is None:  # budget overflow (non-uniform input): grow and retry
        bigger = -(-max_count // 4) * 4
        while True:
            try:
                cfg = Cfg(cfg.vocab, cfg.embed, cfg.ncls, cfg.batch,
                          cfg.doclen, budget=bigger)
                break
            except AssertionError:
                bigger += 4
        gidx, max_count = _prep_index_inputs(cfg, x)

    nc = _get_program(cfg)
    b_tiled = np.tile(b, (128, 1))
    in_maps = []
    for c in range(NCORES):
        in_maps.append({
            "e_sh": embeddings[c * cfg.vsh:(c + 1) * cfg.vsh],
            "w_in": W,
            "b_in": b_tiled,
            "gidx": gidx[c],
        })
    res = run_bass_kernel_spmd(nc, in_maps, list(range(NCORES)),
                               trace=trace, tmpdir=tmpdir)
    out = np.concatenate([res.results[c]["out"] for c in range(NCORES)],
                         axis=0)
    return out, res


def kernel(embeddings, W, b, x):
    out, _ = run(embeddings, W, b, x)
    return out
